# revision 21
# baseline (speedup 1.0000x reference)
"""GNN sparse-attention message passing on 8 Trainium2 NeuronCores.

The axon tunnel (~43MB/s) dominates end-to-end time, so the kernel is
organized around minimizing host<->device bytes and overlapping host work
with the upload:
- k and v ship as ONE int8 table [N, 258] SHARDED (N/8 rows per core):
  k rows per-row-scaled, v globally scaled, and the per-row k scale rides in
  the last 2 bytes of the row as bit-punned fp16. The table is all-gathered
  on device over NeuronLink into a full DRAM table.
- q ships the same way ([N, 130] int8 rows with embedded fp16 scale),
  sharded by the core's fixed destination-node range (local gather); both
  tables ride in ONE [N, 388] int8 array so the whole upload pays a single
  sharded-device_put fixed cost (~40ms) and starts before edge packing.
- Edge indices ship compressed (uint16 src, int8 window offset, uint16 out row
  and group base) and are widened on device by casting gpsimd DMAs; the local
  q row per edge is reconstructed on device as window_offset + group_base.
- Packing the scales into the rows means TWO indirect gathers per edge tile
  (kv row + q row) instead of four — SWDGE descriptor generation on gpsimd is
  the device-side bottleneck (~150ns/row-descriptor).
- Output is quantized on device to int8 with a per-row fp16 scale; the host
  multiplies back (including the global v scale).

Compute (edge parallelism, sharded by destination node):
- Sort edges by dst (uint16 radix argsort); core c owns dst range
  [c*6250, (c+1)*6250).
- Per core, pack edges into groups of G tiles x 128 edges; each group's dst
  nodes lie in a window of <=128 consecutive node ids.
- Per tile: gather kv / q rows per edge via indirect DMA, widen int8->f32 on
  the vector engine; score = exp(clip(kscale*qscale * sum_d k*q / 4));
  msg = v * score.
- One-hot matmul (S_T[e, n] = dst_local[e]==n) accumulates [wV | Z] for the
  group's window in PSUM across the group's tiles; divide, row-quantize, and
  indirect-scatter the window rows to the per-core output slab.
"""
import math

import numpy as np

import concourse.bass as bass
import concourse.tile as tile
from concourse import bacc, mybir

N = 50000
E = 800000
HID = 128
HEADS = 8
HD = 16
NCORES = 8
SH = N // NCORES          # nodes per core (fixed bounds)
MAXN = ((SH + 127) // 128) * 128
G = 12                    # tiles per group
P = 128
KVW = 2 * HID + 2         # kv row: 128 k + 128 v + fp16 k-scale
QW = HID + 2              # q row: 128 q + fp16 q-scale
TW = KVW + QW             # one merged upload row per node
CLIP_LO = float(np.exp(-5.0))
CLIP_HI = float(np.exp(5.0))

_cache = {}


def _pack(e_src, e_dst):
    """Sort edges by dst, shard across fixed core ranges, pack into groups.

    Vectorized: loops only over cores x groups (~500 iterations). dst fits
    uint16, where numpy's stable argsort is a 2-pass radix (~8ms for 800k).
    """
    order = np.argsort(e_dst.astype(np.uint16), kind="stable")
    s = e_src.astype(np.int32)[order]
    d = e_dst.astype(np.int32)[order]
    deg = np.bincount(d, minlength=N)
    cum = np.concatenate([[0], np.cumsum(deg)])  # cum[n] = edges with dst < n

    # greedy group boundaries per core: window <=P nodes, <=G*P edges
    core_groups = []
    for c in range(NCORES):
        n0, n1 = c * SH, (c + 1) * SH
        bases = []
        ni = n0
        while ni < n1:
            bases.append(ni)
            cap_node = min(ni + P, n1)
            cap_edge = int(np.searchsorted(cum, cum[ni] + G * P, side="right")) - 1
            ni = max(ni + 1, min(cap_node, cap_edge))
        core_groups.append(bases)
    Gmax = max(len(b) for b in core_groups)

    per_core = []
    r = np.arange(P)
    for c in range(NCORES):
        n0, n1 = c * SH, (c + 1) * SH
        bases = np.asarray(core_groups[c], np.int64)
        ng = len(bases)
        nxt = np.concatenate([bases[1:], [n1]])
        e0s, e1s = cum[bases], cum[nxt]          # edge ranges per group
        ne = e1s - e0s
        assert int(ne.max(initial=0)) <= G * P, "group edge overflow"

        srcs = np.zeros((Gmax, G * P), np.uint16)
        dstl = np.full((Gmax, G * P), -1, np.int8)
        outr = np.zeros((Gmax, P), np.uint16)
        gbase = np.zeros((Gmax, 1, P), np.uint16)
        trash = (MAXN + r).astype(np.uint16)

        # flat slot index for every edge of this core in one shot
        ce0, ce1 = cum[n0], cum[n1]
        es = s[ce0:ce1]
        ed = d[ce0:ce1]
        slot = np.repeat(np.arange(ng) * (G * P) - (e0s - ce0), ne) \
            + np.arange(ce1 - ce0)
        flat_s = srcs.reshape(-1)
        flat_l = dstl.reshape(-1)
        flat_s[slot] = es.astype(np.uint16)
        flat_l[slot] = (ed - np.repeat(bases, ne)).astype(np.int8)

        span = np.minimum(nxt - bases, P)                       # [ng]
        rows = (bases[:, None] - n0) + r[None, :]               # [ng, P]
        outr[:ng] = np.where(r[None, :] < span[:, None], rows, trash[None, :])
        outr[ng:] = trash[None, :]
        gbase[:ng, 0, :] = (bases[:, None] - n0).astype(np.uint16)

        per_core.append({
            "srcs": np.ascontiguousarray(
                srcs.reshape(Gmax, G, P).transpose(2, 0, 1)).reshape(P, Gmax * G),
            "dstl": np.ascontiguousarray(
                dstl.reshape(Gmax, G, P).transpose(2, 0, 1)).reshape(P, Gmax * G),
            "outr": np.ascontiguousarray(
                outr.reshape(Gmax, 1, P).transpose(2, 0, 1)).reshape(P, Gmax),
            "gbase": np.ascontiguousarray(
                gbase.transpose(2, 0, 1)).reshape(P, Gmax),
            "n0": n0, "n1": n1,
        })
    return per_core, Gmax


def _build(Gmax):
    nc = bacc.Bacc(None, target_bir_lowering=False, num_devices=NCORES)
    f32 = mybir.dt.float32
    f16 = mybir.dt.float16
    i32 = mybir.dt.int32
    i8 = mybir.dt.int8
    u16 = mybir.dt.uint16
    tab = nc.declare_dram_parameter("tab", [SH, TW], i8, isOutput=False)
    srcs = nc.declare_dram_parameter("srcs", [P, Gmax * G], u16, isOutput=False)
    dstl = nc.declare_dram_parameter("dstl", [P, Gmax * G], i8, isOutput=False)
    outr = nc.declare_dram_parameter("outr", [P, Gmax], u16, isOutput=False)
    gbase = nc.declare_dram_parameter("gbase", [P, Gmax], u16, isOutput=False)
    xout = nc.declare_dram_parameter("xout", [MAXN + P, HID + 2], i8, isOutput=True)

    # bounce buffers for the all-gather (collectives can't touch I/O tensors)
    agkv = nc.dram_tensor("agkv", [SH, KVW], i8)
    kvfull = nc.dram_tensor("kvfull", [N, KVW], i8)
    qfull = nc.dram_tensor("qfull", [SH, QW], i8)

    with tile.TileContext(nc) as tc:
        with tc.tile_pool(name="const", bufs=1) as cp, \
             tc.tile_pool(name="sbuf", bufs=3) as sb, \
             tc.tile_pool(name="meta", bufs=2) as mp, \
             tc.tile_pool(name="psum", bufs=2, space="PSUM") as ps:
            nc.sync.dma_start(out=agkv[:], in_=tab[:, :KVW])
            nc.sync.dma_start(out=qfull[:], in_=tab[:, KVW:])
            nc.gpsimd.collective_compute(
                "AllGather", mybir.AluOpType.bypass,
                replica_groups=[list(range(NCORES))],
                ins=[agkv[:].opt()], outs=[kvfull[:].opt()])

            ii = cp.tile([P, P], i32)
            nc.gpsimd.iota(ii[:], pattern=[[1, P]], base=0, channel_multiplier=0)
            fiota = cp.tile([P, P], f32)
            nc.vector.tensor_copy(out=fiota[:], in_=ii[:])

            for g in range(Gmax):
                srcs_sb = mp.tile([P, G], i32, tag="srcs")
                nc.gpsimd.dma_start(out=srcs_sb[:], in_=srcs[:, g * G:(g + 1) * G])
                dstlf_sb = mp.tile([P, G], f32, tag="dstlf")
                nc.gpsimd.dma_start(out=dstlf_sb[:], in_=dstl[:, g * G:(g + 1) * G])
                dstli_sb = mp.tile([P, G], i32, tag="dstli")
                nc.gpsimd.dma_start(out=dstli_sb[:], in_=dstl[:, g * G:(g + 1) * G])
                outr_sb = mp.tile([P, 1], i32, tag="outr")
                nc.gpsimd.dma_start(out=outr_sb[:], in_=outr[:, g:g + 1])
                gb_sb = mp.tile([P, 1], i32, tag="gb")
                nc.gpsimd.dma_start(out=gb_sb[:], in_=gbase[:, g:g + 1])

                # local q row per edge = window offset + group base, clamped >=0
                dstq_sb = mp.tile([P, G], i32, tag="dstq")
                nc.vector.tensor_tensor(
                    out=dstq_sb[:], in0=dstli_sb[:],
                    in1=gb_sb[:].to_broadcast([P, G]), op=mybir.AluOpType.add)
                nc.vector.tensor_scalar(out=dstq_sb[:], in0=dstq_sb[:],
                                        scalar1=0, scalar2=None,
                                        op0=mybir.AluOpType.max)

                acc = ps.tile([P, HID + HEADS], f32, space="PSUM", tag="acc")
                for t in range(G):
                    kvt8 = sb.tile([P, KVW], i8, tag="kvt8")
                    nc.gpsimd.indirect_dma_start(
                        out=kvt8[:], out_offset=None, in_=kvfull[:],
                        in_offset=bass.IndirectOffsetOnAxis(ap=srcs_sb[:, t:t + 1], axis=0))
                    qt8 = sb.tile([P, QW], i8, tag="qt8")
                    nc.gpsimd.indirect_dma_start(
                        out=qt8[:], out_offset=None, in_=qfull[:],
                        in_offset=bass.IndirectOffsetOnAxis(ap=dstq_sb[:, t:t + 1], axis=0))

                    kvf = sb.tile([P, 2 * HID], f32, tag="kvf")
                    nc.vector.tensor_copy(out=kvf[:], in_=kvt8[:, :2 * HID])
                    qef = sb.tile([P, HID], f32, tag="qef")
                    nc.vector.tensor_copy(out=qef[:], in_=qt8[:, :HID])
                    ssc = sb.tile([P, 2], f32, tag="ssc")
                    nc.vector.tensor_copy(
                        out=ssc[:, 0:1],
                        in_=kvt8[:, 2 * HID:2 * HID + 2].bitcast(f16))
                    nc.vector.tensor_copy(
                        out=ssc[:, 1:2], in_=qt8[:, HID:HID + 2].bitcast(f16))

                    st = sb.tile([P, P], f32, tag="st")
                    nc.vector.tensor_tensor(
                        out=st[:], in0=dstlf_sb[:, t:t + 1].to_broadcast([P, P]),
                        in1=fiota[:], op=mybir.AluOpType.is_equal)

                    prod = sb.tile([P, HID], f32, tag="prod")
                    nc.vector.tensor_tensor(
                        out=prod[:], in0=kvf[:, :HID], in1=qef[:],
                        op=mybir.AluOpType.mult)
                    sc = sb.tile([P, HEADS], f32, tag="sc")
                    nc.vector.tensor_reduce(
                        out=sc[:], in_=prod[:].rearrange("p (h d) -> p h d", h=HEADS),
                        axis=mybir.AxisListType.X, op=mybir.AluOpType.add)
                    # apply per-src k scale * per-dst q scale before exp
                    sscp = sb.tile([P, 1], f32, tag="sscp")
                    nc.vector.tensor_tensor(
                        out=sscp[:], in0=ssc[:, 0:1], in1=ssc[:, 1:2],
                        op=mybir.AluOpType.mult)
                    nc.vector.tensor_tensor(
                        out=sc[:], in0=sc[:], in1=sscp[:].to_broadcast([P, HEADS]),
                        op=mybir.AluOpType.mult)
                    nc.scalar.activation(
                        out=sc[:], in_=sc[:],
                        func=mybir.ActivationFunctionType.Exp, scale=1.0 / math.sqrt(HD))
                    msgext = sb.tile([P, HID + HEADS], f32, tag="msgext")
                    nc.vector.tensor_scalar(
                        out=msgext[:, HID:], in0=sc[:],
                        scalar1=CLIP_LO, scalar2=CLIP_HI,
                        op0=mybir.AluOpType.max, op1=mybir.AluOpType.min)
                    nc.vector.tensor_tensor(
                        out=msgext[:, :HID].rearrange("p (h d) -> p h d", h=HEADS),
                        in0=kvf[:, HID:].rearrange("p (h d) -> p h d", h=HEADS),
                        in1=msgext[:, HID:][:, :, None].to_broadcast([P, HEADS, HD]),
                        op=mybir.AluOpType.mult)
                    nc.tensor.matmul(out=acc[:], lhsT=st[:], rhs=msgext[:],
                                     start=(t == 0), stop=(t == G - 1))

                zr = sb.tile([P, HEADS], f32, tag="zr")
                nc.vector.tensor_scalar(out=zr[:], in0=acc[:, HID:], scalar1=1e-6,
                                        scalar2=None, op0=mybir.AluOpType.add)
                nc.vector.reciprocal(out=zr[:], in_=zr[:])
                xsb = sb.tile([P, HID], f32, tag="xsb")
                nc.vector.tensor_tensor(
                    out=xsb[:].rearrange("p (h d) -> p h d", h=HEADS),
                    in0=acc[:, :HID].rearrange("p (h d) -> p h d", h=HEADS),
                    in1=zr[:][:, :, None].to_broadcast([P, HEADS, HD]),
                    op=mybir.AluOpType.mult)

                # per-row int8 quantization: scale = absmax/127, guarded vs 0
                xab = sb.tile([P, HID], f32, tag="xab")
                nc.scalar.activation(out=xab[:], in_=xsb[:],
                                     func=mybir.ActivationFunctionType.Abs)
                rmax = sb.tile([P, 1], f32, tag="rmax")
                nc.vector.tensor_reduce(
                    out=rmax[:], in_=xab[:],
                    axis=mybir.AxisListType.X, op=mybir.AluOpType.max)
                nc.vector.tensor_scalar(out=rmax[:], in0=rmax[:], scalar1=1e-30,
                                        scalar2=None, op0=mybir.AluOpType.add)
                rinv = sb.tile([P, 1], f32, tag="rinv")
                nc.vector.reciprocal(out=rinv[:], in_=rmax[:])
                nc.vector.tensor_scalar(out=rinv[:], in0=rinv[:], scalar1=127.0,
                                        scalar2=None, op0=mybir.AluOpType.mult)
                xq8 = sb.tile([P, HID + 2], i8, tag="xq8")
                nc.vector.tensor_tensor(
                    out=xq8[:, :HID], in0=xsb[:], in1=rinv[:].to_broadcast([P, HID]),
                    op=mybir.AluOpType.mult)
                nc.vector.tensor_scalar(
                    out=xq8[:, HID:HID + 2].bitcast(f16), in0=rmax[:],
                    scalar1=1.0 / 127.0, scalar2=None, op0=mybir.AluOpType.mult)

                nc.gpsimd.indirect_dma_start(
                    out=xout[:], out_offset=bass.IndirectOffsetOnAxis(
                        ap=outr_sb[:, 0:1], axis=0),
                    in_=xq8[:], in_offset=None)
    nc.finalize()
    return nc


def _make_runner(nc):
    """Cached PJRT runner: jitted shard_map over 8 cores with device-created
    donated zero output buffers (avoids uploading zeros over the tunnel)."""
    import jax
    import jax.numpy as jnp
    from jax.experimental.shard_map import shard_map
    from jax.sharding import Mesh, PartitionSpec, NamedSharding
    from concourse.bass2jax import (
        _bass_exec_p, install_neuronx_cc_hook, partition_id_tensor)

    install_neuronx_cc_hook()
    partition_name = nc.partition_id_tensor.name if nc.partition_id_tensor else None

    in_names, out_names, out_avals = [], [], []
    for alloc in nc.m.functions[0].allocations:
        if not isinstance(alloc, mybir.MemoryLocationSet):
            continue
        name = alloc.memorylocations[0].name
        if alloc.kind == "ExternalInput":
            if name != partition_name:
                in_names.append(name)
        elif alloc.kind == "ExternalOutput":
            shape = tuple(alloc.tensor_shape)
            dtype = mybir.dt.np(alloc.dtype)
            out_names.append(name)
            out_avals.append(jax.core.ShapedArray(shape, dtype))

    n_params = len(in_names)
    n_outs = len(out_names)
    all_names = list(in_names) + list(out_names)
    if partition_name is not None:
        all_names.append(partition_name)
    donate = tuple(range(n_params, n_params + n_outs))

    def _body(*args):
        operands = list(args)
        if partition_name is not None:
            operands.append(partition_id_tensor())
        outs = _bass_exec_p.bind(
            *operands,
            out_avals=tuple(out_avals),
            in_names=tuple(all_names),
            out_names=tuple(out_names),
            lowering_input_output_aliases=(),
            sim_require_finite=True,
            sim_require_nnan=True,
            nc=nc,
        )
        return tuple(outs)

    devices = jax.devices()[:NCORES]
    mesh = Mesh(np.asarray(devices), ("core",))
    in_specs = (PartitionSpec("core"),) * (n_params + n_outs)
    out_specs = (PartitionSpec("core"),) * n_outs
    sharded = jax.jit(
        shard_map(_body, mesh=mesh, in_specs=in_specs, out_specs=out_specs,
                  check_rep=False),
        donate_argnums=donate, keep_unused=True)

    zspec = NamedSharding(mesh, PartitionSpec("core"))
    zshapes = [(NCORES * a.shape[0], *a.shape[1:]) for a in out_avals]
    zdtypes = [a.dtype for a in out_avals]
    zeros_fn = jax.jit(
        lambda: tuple(jnp.zeros(s, d) for s, d in zip(zshapes, zdtypes)),
        out_shardings=tuple(zspec for _ in out_avals))

    def run(concat_in_map):
        ins = [concat_in_map[name] for name in in_names]
        zeros = _cache.pop("zeros_next", None)
        if zeros is None:
            zeros = zeros_fn()
        outs = sharded(*ins, *zeros)
        # prefetch donated zero buffers for the next call (device-side, async)
        _cache["zeros_next"] = zeros_fn()
        return {name: outs[i] for i, name in enumerate(out_names)}

    return run, zspec


def kernel(q, k, v, edge_index):
    import jax
    q = np.asarray(q, np.float32).reshape(N, HID)
    k = np.asarray(k, np.float32).reshape(N, HID)
    v = np.asarray(v, np.float32).reshape(N, HID)
    e = np.asarray(edge_index)

    pack0 = None
    if "runner" not in _cache:
        pack0 = _pack(e[0], e[1])
        nc = _build(pack0[1])
        _cache["runner"] = (_make_runner(nc), pack0[1])
    (run, zspec), Gmax_built = _cache["runner"]

    # one merged node table [N, 388] -> single sharded device_put (a sharded
    # put costs one ~40ms fixed overhead + bytes/45MBps on the tunnel)
    # absmax via two reductions (no 25MB abs temp); round-to-nearest via
    # +0.5 offset into the uint8 domain (float->int cast truncates toward 0,
    # which is floor for positives), then shift back to signed
    buf = _cache.get("qbuf")
    if buf is None:
        buf = _cache["qbuf"] = np.empty((N, HID), np.float32)
    ubuf = _cache.get("ubuf")
    if ubuf is None:
        ubuf = _cache["ubuf"] = np.empty((N, HID), np.uint8)
    tab = _cache.get("tab")
    if tab is None:
        tab = _cache["tab"] = np.empty((N, TW), np.int8)
    tabu = tab.view(np.uint8)

    def quant_rows(x, dst_lo):
        scale = (np.maximum(x.max(axis=1), -x.min(axis=1))
                 .reshape(N, 1) * (1.0 / 127.0) + 1e-30)
        np.divide(x, scale, out=buf)
        np.add(buf, 128.5, out=buf)
        ubuf[:] = buf               # trunc == floor: values in [1.5, 255.5)
        np.bitwise_xor(ubuf, 0x80, out=ubuf)   # uint8(x+128) -> int8 bits
        tabu[:, dst_lo:dst_lo + HID] = ubuf
        return scale

    gscale = float(max(v.max(), -v.min())) / 127.0
    np.multiply(v, 1.0 / gscale, out=buf)
    np.add(buf, 128.5, out=buf)
    ubuf[:] = buf
    np.bitwise_xor(ubuf, 0x80, out=ubuf)
    tabu[:, HID:2 * HID] = ubuf
    kscale = quant_rows(k, 0)
    tab[:, 2 * HID:KVW] = kscale.astype(np.float16).view(np.int8)
    qscale = quant_rows(q, KVW)
    tab[:, KVW + HID:] = qscale.astype(np.float16).view(np.int8)
    tab_dev = jax.device_put(tab, zspec)

    # edge packing (overlaps with the async table upload)
    per_core, Gmax = pack0 if pack0 is not None else _pack(e[0], e[1])
    if Gmax != Gmax_built:   # unexpected input distribution: rebuild
        nc = _build(Gmax)
        _cache["runner"] = (_make_runner(nc), Gmax)
        (run, zspec), Gmax_built = _cache["runner"]

    concat = {
        "tab": tab_dev,
        "srcs": np.concatenate([pc["srcs"] for pc in per_core], axis=0),
        "dstl": np.concatenate([pc["dstl"] for pc in per_core], axis=0),
        "outr": np.concatenate([pc["outr"] for pc in per_core], axis=0),
        "gbase": np.concatenate([pc["gbase"] for pc in per_core], axis=0),
    }
    outs = run(concat)

    xall = np.asarray(outs["xout"]).reshape(NCORES, MAXN + P, HID + 2)
    xs = np.ascontiguousarray(xall[:, :SH, HID:]).view(np.float16) \
        .astype(np.float32)
    out = xall[:, :SH, :HID] * (xs * gscale)   # int8 * f32 promotes in one pass
    return out.astype(np.float32, copy=False).reshape(1, N, HID)
